# revision 8
# baseline (speedup 1.0000x reference)
"""GCN conv (normalized-adjacency message passing) on 8 Trainium2 NeuronCores.

Math (matches the dense reference):
    adj      = scatter_set(edge_index, 1.0) + I        (duplicate edges collapse)
    degree   = adj.sum(axis=1)
    dinv     = degree ** -0.5                          (degree >= 1 always, via +I)
    out      = (dinv[:,None] * adj * dinv[None,:]) @ x @ W.T + b

Sharding: 1D node partition, 1024 rows per core. The adjacency is never
materialized. Host-side prep canonicalizes the edge list (dedupe to set
semantics + append the +I self loops = the same graph, different encoding),
sorts it by source row, and pads each 128-row window to a fixed tile count so
all 8 cores run one SPMD program. On device, each core:
  P1  builds per-window one-hot selection matrices Sel[e, r] = (src_e == r)
      with ONE strided DVE compare per window (src free-broadcast against a
      tiled iota), and accumulates degree rows on the PE with a stationary
      ones vector; dinv = sqrt(1/degree).
  AG  AllGathers the 1024-row dinv shards into the full 8192 dinv vector.
  P2  scales x rows: xd = dinv * x (bf16, padded to 128 ch), written to DRAM.
  P3  dma_gathers xd[dst_e] rows (<=1024 idxs per gather, multiple SWDGE
      queues) and accumulates h_w = sum_e Sel_e^T xd[dst_e] per 128-row window
      on the PE, then applies the row scale, the 64x64 linear and bias.
"""

import os

import numpy as np

N = 8192
C = 64
N_CORES = 8
P = 128                      # partitions / window size / edge-tile size
ROWS_PER_CORE = N // N_CORES  # 1024
W_PER_CORE = ROWS_PER_CORE // P  # 8 windows of 128 rows per core
N_WIN = N // P               # 64 windows total
GT = 8                       # tiles per dma_gather (1024-descriptor ring cap)
N_QUEUES = 4                 # SWDGE queues to spread gathers over

_cache = {}


def _prep(edge_index):
    """Canonicalize + shard the edge list.

    Returns (T, src_bf16, dst_i16):
      T        tiles (of 128 edges) per 128-row window, uniform across cores
      src_bf16 [8, 128, 8*T]   row-in-window (src % 128) per edge, -1 padding
      dst_i16  [8, 128, 8*T*8] gather indices in dma_gather's 16-row wrap
                               layout, replicated across the 8 Q7 groups
    """
    import ml_dtypes

    src = edge_index[0].astype(np.int64)
    dst = edge_index[1].astype(np.int64)
    keys = np.unique(src * N + dst)          # set semantics: dups collapse
    us = keys // N
    ud = keys % N
    # + identity: one extra self edge per node (kept separate from any real
    # self edge so the diagonal sums to 2.0 when both exist, as in adj + I)
    us = np.concatenate([us, np.arange(N, dtype=np.int64)])
    ud = np.concatenate([ud, np.arange(N, dtype=np.int64)])
    order = np.argsort(us, kind="stable")
    us, ud = us[order], ud[order]

    win = us // P                            # 64 windows of 128 rows
    cnt = np.bincount(win, minlength=N_WIN)
    T = int(max(3, -(-cnt.max() // P)))      # tiles per window (uniform)
    ew = T * P                               # padded edges per window

    src_pad = np.full((N_WIN, ew), -1.0, np.float32)   # -1 never matches iota
    dst_pad = np.zeros((N_WIN, ew), np.int16)          # pad gathers row 0
    bounds = np.concatenate([[0], np.cumsum(cnt)])
    for w in range(N_WIN):
        lo, hi = bounds[w], bounds[w + 1]
        # sort the window's edges by destination: the gather then walks xd
        # mostly sequentially (HBM row-buffer locality)
        o = np.argsort(ud[lo:hi], kind="stable")
        src_pad[w, : hi - lo] = (us[lo:hi][o] % P).astype(np.float32)
        dst_pad[w, : hi - lo] = ud[lo:hi][o].astype(np.int16)

    # src: edge j of a window sits at SBUF [j % 128, w*T + j//128]
    a = src_pad.reshape(N_CORES, W_PER_CORE, T, P)
    src_bf16 = np.ascontiguousarray(
        a.transpose(0, 3, 1, 2).reshape(N_CORES, P, W_PER_CORE * T)
    ).astype(ml_dtypes.bfloat16)
    # dst: dma_gather reads idx j from [j % 16, j // 16] of the window slice,
    # replicated into all 8 16-partition groups (one per Q7 gpsimd core)
    d = dst_pad.reshape(N_CORES, W_PER_CORE, T * 8, 16)
    wrap = d.transpose(0, 3, 1, 2).reshape(N_CORES, 16, W_PER_CORE * T * 8)
    dst_i16 = np.tile(wrap, (1, 8, 1))
    return T, src_bf16, dst_i16


def _build(T):
    """Build + compile the SPMD Bass program for tile capacity T."""
    from contextlib import ExitStack

    import concourse.bacc as bacc
    import concourse.mybir as mybir
    import concourse.tile as tile
    from concourse.masks import make_identity

    f32 = mybir.dt.float32
    bf16 = mybir.dt.bfloat16
    i16 = mybir.dt.int16
    NJ = N // P  # 64 column chunks of x

    nc = bacc.Bacc(
        "TRN2",
        target_bir_lowering=False,
        debug=False,
        num_devices=N_CORES,
        num_swdge_queues=N_QUEUES,
    )
    x_in = nc.dram_tensor("x", [P, NJ * C], f32, kind="ExternalInput").ap()
    src_in = nc.dram_tensor(
        "srcf", [P, W_PER_CORE * T], bf16, kind="ExternalInput"
    ).ap()
    dst_in = nc.dram_tensor(
        "dsti", [P, W_PER_CORE * T * 8], i16, kind="ExternalInput"
    ).ap()
    iota_in = nc.dram_tensor("iota", [P, P], bf16, kind="ExternalInput").ap()
    wt_in = nc.dram_tensor("wt", [C, C], f32, kind="ExternalInput").ap()
    bt_in = nc.dram_tensor("bt", [P, C], f32, kind="ExternalInput").ap()
    out_t = nc.dram_tensor(
        "out", [ROWS_PER_CORE, C], f32, kind="ExternalOutput"
    ).ap()

    with tile.TileContext(nc) as tc, ExitStack() as ctx:
        dram = ctx.enter_context(tc.tile_pool(name="dram", bufs=1, space="DRAM"))
        const = ctx.enter_context(tc.tile_pool(name="const", bufs=1))
        sb = ctx.enter_context(tc.tile_pool(name="sb", bufs=3))
        selp = ctx.enter_context(tc.tile_pool(name="selp", bufs=W_PER_CORE))
        gxp = ctx.enter_context(tc.tile_pool(name="gxp", bufs=3))
        psum = ctx.enter_context(tc.tile_pool(name="psum", bufs=2, space="PSUM"))

        xd_d = dram.tile([N, 2 * C], bf16)       # xd padded to 128 ch (256B)
        cc_in = dram.tile([ROWS_PER_CORE, 1], f32)
        cc_out = dram.tile([N, 1], f32)

        # constants
        iota_sb = const.tile([P, P], bf16)
        nc.sync.dma_start(out=iota_sb[:], in_=iota_in[:])
        ones_sb = const.tile([P, 1], bf16)
        nc.vector.memset(ones_sb[:], 1.0)
        ident_sb = const.tile([P, P], f32)
        make_identity(nc, ident_sb[:])
        wt_sb = const.tile([C, C], f32)
        nc.sync.dma_start(out=wt_sb[:], in_=wt_in[:])
        bt_sb = const.tile([P, C], f32)
        nc.sync.dma_start(out=bt_sb[:], in_=bt_in[:])
        srcf_sb = const.tile([P, W_PER_CORE * T], bf16)
        nc.sync.dma_start(out=srcf_sb[:], in_=src_in[:])
        dsti_sb = const.tile([P, W_PER_CORE * T * 8], i16)
        nc.sync.dma_start(out=dsti_sb[:], in_=dst_in[:])

        # P1: per-window Sel (one strided compare) + degree rows on PE
        deg_row = const.tile([1, ROWS_PER_CORE], f32)  # [w*128 + r]
        sel_tiles = []
        for w in range(W_PER_CORE):
            selw = selp.tile([P, T * P], bf16, tag="sel")
            sel_tiles.append(selw)
            nc.vector.tensor_tensor(
                out=selw[:].rearrange("p (t r) -> p t r", r=P),
                in0=srcf_sb[:, w * T : (w + 1) * T, None].to_broadcast(
                    [P, T, P]
                ),
                in1=iota_sb[:, None, :].to_broadcast([P, T, P]),
                op=mybir.AluOpType.is_equal,
            )
            dg_ps = psum.tile([1, P], f32, tag="deg")
            for t in range(T):
                nc.tensor.matmul(
                    dg_ps[:],
                    lhsT=ones_sb[:],
                    rhs=selw[:, t * P : (t + 1) * P],
                    start=(t == 0),
                    stop=(t == T - 1),
                )
            nc.vector.tensor_copy(
                deg_row[:, w * P : (w + 1) * P], dg_ps[:]
            )
        # dinv = sqrt(1/degree) on the single row, then AllGather
        rec_row = sb.tile([1, ROWS_PER_CORE], f32, tag="rec")
        nc.vector.reciprocal(rec_row[:], deg_row[:])
        dinv_row = sb.tile([1, ROWS_PER_CORE], f32, tag="dinvr")
        nc.scalar.sqrt(dinv_row[:], rec_row[:])
        nc.sync.dma_start(out=cc_in[:, 0][None, :], in_=dinv_row[:])
        nc.gpsimd.collective_compute(
            "AllGather",
            mybir.AluOpType.bypass,
            replica_groups=[list(range(N_CORES))],
            ins=[cc_in.opt()],
            outs=[cc_out.opt()],
        )

        # P2: xd = dinv * x (bf16, zero-padded channels 64..127)
        dinv_n = const.tile([P, NJ], f32)
        nc.sync.dma_start(
            out=dinv_n[:], in_=cc_out[:, 0].rearrange("(j p) -> p j", p=P)
        )
        x_sb = const.tile([P, NJ * C], f32)
        nc.sync.dma_start(out=x_sb[:], in_=x_in[:])
        xd_sb = const.tile([P, NJ * 2 * C], bf16)
        nc.vector.memset(xd_sb[:], 0.0)
        nc.vector.tensor_tensor(
            out=xd_sb[:].rearrange("p (j e) -> p j e", e=2 * C)[:, :, :C],
            in0=x_sb[:].rearrange("p (j e) -> p j e", e=C),
            in1=dinv_n[:, :, None].to_broadcast([P, NJ, C]),
            op=mybir.AluOpType.mult,
        )
        nc.sync.dma_start(
            out=xd_d[:].rearrange("(j p) e -> p j e", p=P),
            in_=xd_sb[:].rearrange("p (j e) -> p j e", e=2 * C),
        )

        # per-window row dinv for the epilogue: [p, w] layout straight from DRAM
        dinv_w = const.tile([P, W_PER_CORE], f32)
        nc.sync.dma_start(
            out=dinv_w[:], in_=cc_in[:, 0].rearrange("(w p) -> p w", p=P)
        )

        # P3: gather + accumulate h per window, then linear + bias
        for w in range(W_PER_CORE):
            gx = gxp.tile([P, T * 2 * C], bf16, tag="gx")
            for gi, g in enumerate(range(0, T, GT)):
                gn = min(GT, T - g)
                nc.gpsimd.dma_gather(
                    out_ap=gx[:, g * 2 * C : (g + gn) * 2 * C].rearrange(
                        "p (t e) -> p t e", e=2 * C
                    ),
                    in_ap=xd_d[:],
                    idxs_ap=dsti_sb[
                        :, (w * T + g) * 8 : (w * T + g + gn) * 8
                    ],
                    num_idxs=gn * P,
                    num_idxs_reg=gn * P,
                    elem_size=2 * C,
                    queue_num=(w * 5 + gi) % N_QUEUES,
                )
            selw = sel_tiles[w]
            h_ps = psum.tile([P, C], f32, tag="h")
            for t in range(T):
                nc.tensor.matmul(
                    h_ps[:],
                    lhsT=selw[:, t * P : (t + 1) * P],
                    rhs=gx[:, t * 2 * C : t * 2 * C + C],
                    start=(t == 0),
                    stop=(t == T - 1),
                )
            hs = sb.tile([P, C], f32, tag="hs")
            nc.vector.tensor_scalar_mul(hs[:], h_ps[:], dinv_w[:, w : w + 1])
            hsT_ps = psum.tile([C, P], f32, tag="hsT")
            nc.tensor.transpose(hsT_ps[:], hs[:], ident_sb[:])
            hsT = sb.tile([C, P], f32, tag="hsTs")
            nc.vector.tensor_copy(hsT[:], hsT_ps[:])
            o_ps = psum.tile([P, C], f32, tag="o")
            nc.tensor.matmul(
                o_ps[:], lhsT=hsT[:], rhs=wt_sb[:], start=True, stop=True
            )
            ob = sb.tile([P, C], f32, tag="ob")
            nc.vector.tensor_add(ob[:], o_ps[:], bt_sb[:])
            nc.sync.dma_start(out=out_t[w * P : (w + 1) * P, :], in_=ob[:])

    nc.compile()
    return nc


def _maybe_trace_hook():
    """Register the axon NTFF profile hook if tracing is requested and the
    container's boot skipped it (antenv.axon_hooks missing)."""
    import sys
    import types

    try:
        import antenv

        if "antenv.axon_hooks" not in sys.modules:
            mod = types.ModuleType("antenv.axon_hooks")
            mod._hook = None
            mod.set_axon_ntff_profile_hook = lambda h: setattr(mod, "_hook", h)
            mod.get_axon_ntff_profile_hook = lambda: mod._hook
            sys.modules["antenv.axon_hooks"] = mod
            antenv.axon_hooks = mod
        mod = sys.modules["antenv.axon_hooks"]
        if mod.get_axon_ntff_profile_hook() is None:
            from trn_agent_boot.trn_boot import _ntff_profile_via_ctypes

            mod.set_axon_ntff_profile_hook(
                _ntff_profile_via_ctypes("/opt/axon/libaxon_pjrt.so")
            )
        return True
    except Exception:
        return False


def kernel(x, edge_index, W, b):
    import ml_dtypes

    from concourse.bass_utils import run_bass_kernel_spmd

    x = np.ascontiguousarray(np.asarray(x, dtype=np.float32))
    W = np.asarray(W, dtype=np.float32)
    b = np.asarray(b, dtype=np.float32)
    T, src_bf16, dst_i16 = _prep(np.asarray(edge_index))

    if T not in _cache:
        _cache[T] = _build(T)
    nc = _cache[T]

    # x in [p, (j c)] layout: row n = j*128 + p
    x_wrap = np.ascontiguousarray(
        x.reshape(N // P, P, C).transpose(1, 0, 2).reshape(P, (N // P) * C)
    )
    iota_np = np.tile(
        np.arange(P, dtype=np.float32), (P, 1)
    ).astype(ml_dtypes.bfloat16)
    wt = np.ascontiguousarray(W.T)
    bt = np.tile(b[None, :], (P, 1))
    in_maps = [
        {
            "x": x_wrap,
            "srcf": src_bf16[c],
            "dsti": dst_i16[c],
            "iota": iota_np,
            "wt": wt,
            "bt": bt,
        }
        for c in range(N_CORES)
    ]

    trace = os.environ.get("BASS_GCN_TRACE", "") == "1" and _maybe_trace_hook()
    res = run_bass_kernel_spmd(
        nc, in_maps, core_ids=list(range(N_CORES)), trace=trace
    )
    if trace and res.exec_time_ns is not None:
        print(f"HW exec time: {res.exec_time_ns} ns")
    return np.concatenate([res.results[c]["out"] for c in range(N_CORES)], axis=0)
